# revision 1
# baseline (speedup 1.0000x reference)
"""Trainium2 Bass kernel for the CMA momentum-memory update (nn_CMA_52956946760162).

Strategy (class-sharded, present-only compact packing):
- Shard the C=4096 classes across 8 cores (512 classes/core), no collectives.
- Host packs, per (core, modality), the *present* (label,cam) segments and
  present labels into chunks of <=128 one-hot columns / <=128*B feature rows
  (whole classes per chunk). The one-hot entries are pre-scaled with the
  momentum/count coefficients (b_c = sigma_or_1/cnt, b_g = sigma/cnt), and a
  segment column and its class column share the same matmul, so one tensor-
  engine pass produces both per-(label,cam) and per-label scaled sums in PSUM.
- Host gathers the corresponding memory-bank rows densely (mem_in), so every
  device DMA is a dense [128 x 2048] f32 block. The device computes
  out = a * mem + psum in a single fused DVE op per chunk and streams it out.
- Rows absent from the batch leave memory unchanged; the host passes them
  through directly from the input banks during output assembly and scatters
  the device-computed rows over them.
"""

import numpy as np

C, K, D, N = 4096, 6, 2048, 16384
SIGMA = 0.2
M = 8                 # cores
CPC = C // M          # classes per core = 512
CK = C * K
F32 = np.float32

_BUILD_CACHE = {}


def _pack_core_modality(core, feats, labels, cams, valid, B, nch):
    """Pack one (core, modality) into chunk tensors.

    Returns fpad [nch*B*128, D], oh [nch, B*128, 128], avec [128, nch],
    mem_idx/out_idx [nch, 128] (merged row id: class c -> c, seg s -> CPC + s,
    pad -> -1).
    """
    c0 = core * CPC
    mask = (labels >= c0) & (labels < c0 + CPC)
    rows_all = np.nonzero(mask)[0]
    lab = labels[rows_all] - c0
    seg = lab * K + cams[rows_all]
    order = np.argsort(seg, kind="stable")
    rows_all, lab, seg = rows_all[order], lab[order], seg[order]

    ccnt = np.bincount(seg, minlength=CPC * K).astype(F32)
    gcnt = np.bincount(lab, minlength=CPC).astype(F32)
    v = np.asarray(valid[c0:c0 + CPC]).reshape(CPC * K)
    a_c = np.where(v, 1.0 - SIGMA, 0.0).astype(F32)
    b_c = (np.where(v, SIGMA, 1.0) / np.maximum(ccnt, 1.0)).astype(F32)
    b_g = (SIGMA / np.maximum(gcnt, 1.0)).astype(F32)

    cpres = ccnt > 0
    class_start = np.searchsorted(lab, np.arange(CPC + 1))
    nseg_per_class = cpres.reshape(CPC, K).sum(axis=1)

    chunk_id = np.empty(len(rows_all), np.int64)
    slot = np.empty(len(rows_all), np.int64)
    segcol_of = np.empty(CPC * K, np.int64)
    ccol_of = np.empty(CPC, np.int64)
    mem_idx = np.full((nch, 128), -1, np.int64)
    out_idx = np.full((nch, 128), -1, np.int64)
    avec = np.zeros((128, nch), F32)

    present = np.nonzero(gcnt > 0)[0]
    chunk_classes = []
    cur, cols, rws = [], 0, 0
    for c in present:
        ns = int(nseg_per_class[c])
        nr = int(class_start[c + 1] - class_start[c])
        if cur and (cols + ns + 1 > 128 or rws + nr > B * 128):
            chunk_classes.append(cur)
            cur, cols, rws = [], 0, 0
        cur.append(c)
        cols += ns + 1
        rws += nr
    if cur:
        chunk_classes.append(cur)
    assert len(chunk_classes) <= nch

    for j, cls_list in enumerate(chunk_classes):
        lo, rws = 0, 0
        for c in cls_list:
            segs_c = np.nonzero(cpres[c * K:(c + 1) * K])[0] + c * K
            for s in segs_c:
                p, lo = lo, lo + 1
                segcol_of[s] = p
                out_idx[j, p] = CPC + s
                avec[p, j] = a_c[s]
                mem_idx[j, p] = CPC + s
            p, lo = lo, lo + 1
            ccol_of[c] = p
            out_idx[j, p] = c
            avec[p, j] = 1.0 - SIGMA
            mem_idx[j, p] = c
            r0, r1 = int(class_start[c]), int(class_start[c + 1])
            chunk_id[r0:r1] = j
            slot[r0:r1] = rws + np.arange(r1 - r0)
            rws += r1 - r0
        assert lo <= 128

    fpoh = np.zeros((nch, B * 128, D + 128), F32)
    fpoh[chunk_id, slot, :D] = feats[rows_all]
    fpoh[chunk_id, slot, D + segcol_of[seg]] = b_c[seg]
    fpoh[chunk_id, slot, D + ccol_of[lab]] = b_g[lab]
    return dict(fpoh=fpoh.reshape(nch * B * 128, D + 128), avec=avec,
                mem_idx=mem_idx, out_idx=out_idx)


def _chunk_stats(labels, cams, valid):
    """Per core: (max rows per class, gcnt, n1_of, n0_of)."""
    out = []
    for core in range(M):
        c0 = core * CPC
        mask = (labels >= c0) & (labels < c0 + CPC)
        lab = labels[mask] - c0
        seg = lab * K + cams[mask]
        gcnt = np.bincount(lab, minlength=CPC)
        cpres = np.bincount(seg, minlength=CPC * K) > 0
        v = np.asarray(valid[c0:c0 + CPC]).reshape(CPC * K)
        vseg = (cpres & v).reshape(CPC, K).sum(axis=1)
        nseg = cpres.reshape(CPC, K).sum(axis=1)
        out.append((int(gcnt.max()), gcnt, vseg + 1, nseg - vseg))
    return out


def _count_chunks(gcnt, n1_of, n0_of, B):
    j, cols, rws, any_rows = 0, 0, 0, False
    for c in np.nonzero(gcnt > 0)[0]:
        ns = int(n1_of[c] + n0_of[c])    # total cols for class c
        nr = int(gcnt[c])
        if any_rows and (cols + ns > 128 or rws + nr > B * 128):
            j += 1
            cols, rws = 0, 0
        cols += ns
        rws += nr
        any_rows = True
    return j + 1 if any_rows else 0


def _build_program(B, nch):
    """Build + compile the SPMD Bass program; 2*nch chunks (both modalities)."""
    import concourse.mybir as mybir
    import concourse.tile as tile
    from concourse import bacc

    f32 = mybir.dt.float32
    nc = bacc.Bacc("TRN2", target_bir_lowering=False, debug=False)

    NT = 2 * nch
    H = D // 2
    fpoh = nc.dram_tensor("fpoh", [NT * B * 128, D + 128], f32, kind="ExternalInput").ap()
    memin = nc.dram_tensor("memin", [NT * 128, D], f32, kind="ExternalInput").ap()
    avec = nc.dram_tensor("avec", [128, NT], f32, kind="ExternalInput").ap()
    out = nc.dram_tensor("out", [NT * 128, D], f32, kind="ExternalOutput").ap()

    with tile.TileContext(nc) as tc:
        with tc.tile_pool(name="const", bufs=1) as constp, \
             tc.tile_pool(name="io", bufs=6) as iop, \
             tc.tile_pool(name="ps", bufs=2, space="PSUM") as psp:

            avec_t = constp.tile([128, NT], f32, name="avec_t")
            nc.sync.dma_start(out=avec_t[:], in_=avec[:, :])

            for j in range(NT):
                psum = psp.tile([128, D], f32, tag="ps", name="psum")
                for b in range(B):
                    r0 = (j * B + b) * 128
                    frow = iop.tile([128, D + 128], f32, tag="frow", name="frow")
                    nc.sync.dma_start(out=frow[:], in_=fpoh[r0:r0 + 128, :])
                    for t in range(4):
                        sl = slice(t * 512, (t + 1) * 512)
                        nc.tensor.matmul(psum[:, sl], frow[:, D:D + 128], frow[:, sl],
                                         start=(b == 0), stop=(b == B - 1))
                mem_sb = iop.tile([128, D], f32, tag="mem", bufs=5, name="mem_sb")
                nc.scalar.dma_start(out=mem_sb[:], in_=memin[j * 128:(j + 1) * 128, :])
                out_sb = iop.tile([128, D], f32, tag="out", bufs=8, name="out_sb")
                nc.vector.scalar_tensor_tensor(
                    out=out_sb[:], in0=mem_sb[:], scalar=avec_t[:, j:j + 1],
                    in1=psum[:], op0=mybir.AluOpType.mult, op1=mybir.AluOpType.add)
                nc.gpsimd.dma_start(out=out[j * 128:(j + 1) * 128, :], in_=out_sb[:])

    nc.compile()
    return nc


def prepare(inputs):
    """Build (or reuse) the program and the per-core input maps + scatter metadata."""
    a = {k: np.ascontiguousarray(np.asarray(v)) for k, v in inputs.items()}
    mods = [
        (a["rgb_feats"], a["rgb_labels"].astype(np.int64), a["rgb_cams"].astype(np.int64),
         a["vis_cam_valid"], a["vis_memory"], a["vis_cam_memory"].reshape(CK, D)),
        (a["ir_feats"], a["ir_labels"].astype(np.int64), a["ir_cams"].astype(np.int64),
         a["ir_cam_valid"], a["ir_memory"], a["ir_cam_memory"].reshape(CK, D)),
    ]

    # global B and chunk count (uniform across cores -> one SPMD program)
    B = 1
    stats = []
    for feats, labels, cams, valid, gmem, cmem in mods:
        st = _chunk_stats(labels, cams, valid)
        stats.append(st)
        for mx, _, _, _ in st:
            B = max(B, int(np.ceil(mx / 128)))
    nch = 1
    for st in stats:
        for _, gcnt, n1_of, n0_of in st:
            nch = max(nch, _count_chunks(gcnt, n1_of, n0_of, B))

    key = (B, nch)
    if key not in _BUILD_CACHE:
        _BUILD_CACHE[key] = _build_program(B, nch)
    nc = _BUILD_CACHE[key]

    in_maps, metas = [], []
    for core in range(M):
        c0 = core * CPC
        packs = []
        for m, (feats, labels, cams, valid, gmem, cmem) in enumerate(mods):
            packs.append(_pack_core_modality(core, feats, labels, cams, valid, B, nch))
        im = {
            "fpoh": np.concatenate([p["fpoh"] for p in packs], axis=0),
            "avec": np.concatenate([p["avec"] for p in packs], axis=1),
        }
        memin = np.zeros((2 * nch * 128, D), F32)
        meta = []
        for m, p in enumerate(packs):
            gmem, cmem = mods[m][4], mods[m][5]
            idx = p["mem_idx"].reshape(nch * 128)
            used = np.nonzero(idx >= 0)[0]
            gidx = idx[used]
            isg = gidx < CPC
            src = np.where(isg, c0 + gidx, core * CPC * K + (gidx - CPC))
            block = memin[m * nch * 128:(m + 1) * nch * 128]
            block[used[isg]] = gmem[src[isg]]
            block[used[~isg]] = cmem[src[~isg]]
            oidx = p["out_idx"].reshape(nch * 128)
            oused = np.nonzero(oidx >= 0)[0]
            ogidx = oidx[oused]
            oisg = ogidx < CPC
            obase = (C + CK) * m
            tgt = np.where(oisg, obase + c0 + ogidx,
                           obase + C + core * CPC * K + (ogidx - CPC))
            meta.append((oused + m * nch * 128, tgt))
        im["memin"] = memin
        in_maps.append(im)
        metas.append(meta)
    return nc, in_maps, metas, a, mods


def assemble(a, mods, metas, results):
    full = np.concatenate([a["vis_memory"], mods[0][5], a["ir_memory"], mods[1][5]],
                          axis=0).astype(F32, copy=True)
    for core in range(M):
        o = results[core]["out"]
        for used, tgt in metas[core]:
            full[tgt] = o[used]
    return full


def kernel(**inputs):
    from concourse.bass_utils import run_bass_kernel_spmd

    nc, in_maps, metas, a, mods = prepare(inputs)
    res = run_bass_kernel_spmd(nc, in_maps, core_ids=list(range(M)))
    return assemble(a, mods, metas, res.results)

